# revision 2
# baseline (speedup 1.0000x reference)
"""Trainium2 Bass kernel: GRU encoder-decoder with Bahdanau attention.

Model: B=4096, T=56 enc steps, S=28 dec steps, H=126.
Sharding: pure data parallel, batch 4096 -> 8 cores x 512.

v2 design vs baseline:
  - all GRU/Wh/Uo matmuls in f32r (bitcast views; 1 cyc/row vs fp32 4)
  - biases folded into matmuls via ones-rows (h stored as [127,256])
  - softmax without max-subtraction, V_b dropped (softmax-invariant)
  - batch split into 2 halves of 256, pipelined against each other
  - attention elementwise work split DVE (c-chunk 0) / Pool (c-chunk 1)
  - scores PSUM evacuated by strided DMA straight into batch-partitioned
    SBUF (no ACT copy)
"""
import sys
import numpy as np

for _p in ('/opt/trn_rl_repo', '/root/.axon_site/_ro/trn_rl_repo'):
    if _p not in sys.path:
        sys.path.insert(0, _p)

from concourse import bass, tile
from concourse.vector_clock import ScopedClock

mybir = bass.mybir
F32 = mybir.dt.float32
F32R = mybir.dt.float32r
BF16 = mybir.dt.bfloat16
AF = mybir.ActivationFunctionType
ALU = mybir.AluOpType
AX = mybir.AxisListType

# ---- workaround: this walrus build allows only one embedded sync-wait on
# the Tile tail drain; spread the global-clock waits over SP nops instead.
def _patched_drain_and_barrier(self, tick_clock, wait_clock):
    nc = self.nc
    probe = nc.sync.nop()
    wait_clock.add_sem_waits(probe.ins, ScopedClock({None: tick_clock.global_clock}))
    si = probe.ins.sync_info
    waits = list(si.on_wait or []) if si is not None else []
    if si is not None:
        si.on_wait = waits[:1]
    for w in waits[1:]:
        n2 = nc.sync.nop()
        n2.ins.sync_info = mybir.SyncInfo(on_wait=[w], on_update=[])
    nc.sync.drain()
    nc.all_engine_barrier()
    popped = nc._tile_sem_poison_stack.pop()
    assert popped is self._sem_poison
    nc.clear_and_free_semaphores(list(self.sems.allocated().values()))
    nc.all_engine_barrier()

tile.TileContext._drain_and_barrier = _patched_drain_and_barrier


def _split_excess_waits(nc):
    """This walrus build allows 1 embedded sync-wait per instruction; move
    extras onto same-engine nops inserted just before the instruction."""
    cnt = 0
    for _, bassbb in list(nc.bb_map.items()):
        bb = bassbb.bb if hasattr(bassbb, "bb") else bassbb
        il = bb.instructions
        i = 0
        while i < len(il):
            inst = il[i]
            si = inst.sync_info
            if si is not None and si.on_wait and len(si.on_wait) > 1:
                extra = list(si.on_wait[:-1])
                si.on_wait = [si.on_wait[-1]]
                for w in extra:
                    cnt += 1
                    nop = mybir.InstNoOp(name=f"wfix-{cnt}", ins=[], outs=[])
                    nop.engine = inst.engine
                    nop.sync_info = mybir.SyncInfo(on_wait=[w], on_update=[])
                    il.insert(i, nop)
                    i += 1
            i += 1
    return cnt

B, T, S = 4096, 56, 28
H, ANN, ENC, DEC = 126, 30, 20, 15
NCORES = 8
BS = B // NCORES          # 512 batch per core
BH = BS // 2              # 256 per half
TQS = 8                   # t-block for the scores pipeline
NQ = T // TQS             # 7
BBLK = 32                 # batch block per scores matmul

_CACHE = {}


def _build_program():
    import os
    kt = int(os.environ.get("K_T", T))
    ks = int(os.environ.get("K_S", S))
    k_xqd = int(os.environ.get("K_XQD", 0))    # xq blocks on DVE per half
    k_merge = os.environ.get("K_MERGE", "dve")  # P1+=P2 engine
    k_order = int(os.environ.get("K_ORDER", 0))
    nc = bass.Bass()
    pool_eng = nc.gpsimd if k_pool else nc.vector

    di = lambda name, shape: nc.declare_dram_parameter(name, list(shape), BF16, isOutput=False)
    enc_d = di("enc", (T, ENC, BS))
    dec_d = di("dec", (S, DEC, BS))
    ann_d = di("ann", (ANN, BS))
    W1a_d = di("W1a", (ANN + 1, 96))
    W2a_d = di("W2a", (97, H))
    Wiea_d = di("Wiea", (ENC + 1, 3 * H))
    Whea_d = di("Whea", (H + 1, 3 * H))
    Wda_d = di("Wda", (DEC + 2, 3 * H))
    WihA_d = di("WihA", (H, 3 * H))
    Whda_d = di("Whda", (H + 1, 3 * H))
    UWa_d = di("UWa", (H + 1, H))
    WlWa_d = di("WlWa", (H + 1, H))
    h2oa_d = di("h2oa", (H + 1, 1))
    Vw_d = di("Vw", (H, 1))
    id_d = di("ident", (128, 128))
    ones_d = di("ones", (1, BS))
    out_d = nc.declare_dram_parameter("out", [S, BS], BF16, isOutput=True)

    from contextlib import ExitStack
    with tile.TileContext(nc) as tc, ExitStack() as es:
        cp = es.enter_context(tc.tile_pool(name="const", bufs=1))
        sp = es.enter_context(tc.tile_pool(name="sb", bufs=2))
        gp = es.enter_context(tc.tile_pool(name="gates", bufs=2))
        pq = es.enter_context(tc.tile_pool(name="pq", bufs=1))
        gq = es.enter_context(tc.tile_pool(name="gq", bufs=5))
        ppg = es.enter_context(tc.tile_pool(name="psg", bufs=2, space="PSUM"))
        ppsc = es.enter_context(tc.tile_pool(name="pssc", bufs=2, space="PSUM"))
        pptr = es.enter_context(tc.tile_pool(name="pstr", bufs=1, space="PSUM"))

        def cload(dram, shape, dtype=BF16):
            t_ = cp.tile(list(shape), dtype, tag="c_" + dram.name)
            nc.sync.dma_start(out=t_[:], in_=dram[:])
            return t_

        W1a = cload(W1a_d, (ANN + 1, 96))
        W2a = cload(W2a_d, (97, H))
        Wiea = cload(Wiea_d, (ENC + 1, 3 * H))
        Whea = cload(Whea_d, (H + 1, 3 * H))
        Wda = cload(Wda_d, (DEC + 2, 3 * H))
        WihA = cload(WihA_d, (H, 3 * H))
        Whda = cload(Whda_d, (H + 1, 3 * H))
        UWa = cload(UWa_d, (H + 1, H))
        WlWa = cload(WlWa_d, (H + 1, H))
        h2oa = cload(h2oa_d, (H + 1, 1))
        Vb = cload(Vw_d, (H, 1))
        idb = cload(id_d, (128, 128))
        WihAb = WihA

        r32 = lambda ap: ap  # all matmul operands are bf16

        # persistent big tensors
        Uo = cp.tile([H, T, BS], BF16, tag="Uo")            # 57.3 KB/part
        encb = cp.tile([128, 4, H, T], BF16, tag="encb")    # 56.4 KB/part

        # manual ring buffers with preset ones-rows
        hbuf = [[cp.tile([H, BH], F32, tag=f"h{g}{k}", name=f"h{g}{k}") for k in (0, 1)]
                for g in (0, 1)]
        hbb = [[cp.tile([H + 1, BH], BF16, tag=f"hb{g}{k}", name=f"hb{g}{k}") for k in (0, 1)]
               for g in (0, 1)]
        xtb = [[cp.tile([ENC + 1, BH], BF16, tag=f"xt{g}{k}", name=f"xt{g}{k}") for k in (0, 1)]
               for g in (0, 1)]
        dxb = [[cp.tile([DEC + 2, BH], BF16, tag=f"dx{g}{k}", name=f"dx{g}{k}") for k in (0, 1)]
               for g in (0, 1)]
        hid = [cp.tile([97, BH], BF16, tag=f"hid{g}", name=f"hid{g}") for g in (0, 1)]
        anb = [cp.tile([ANN + 1, BH], BF16, tag=f"an{g}", name=f"an{g}") for g in (0, 1)]
        ones_h = ones_d[:, 0:BH]
        for g in (0, 1):
            for k in (0, 1):
                nc.sync.dma_start(out=hbb[g][k][H:H + 1, :], in_=ones_h)
                nc.sync.dma_start(out=xtb[g][k][ENC:ENC + 1, :], in_=ones_h)
                nc.sync.dma_start(out=dxb[g][k][DEC + 1:DEC + 2, :], in_=ones_h)
            nc.sync.dma_start(out=hid[g][96:97, :], in_=ones_h)
            nc.sync.dma_start(out=anb[g][ANN:ANN + 1, :], in_=ones_h)

        gsl = lambda g: slice(g * BH, (g + 1) * BH)

        # ---------------- phase A: static -> h0 ----------------
        for g in (0, 1):
            nc.sync.dma_start(out=anb[g][0:ANN, :], in_=ann_d[:, gsl(g)])
            ps96 = ppg.tile([H, 2, BH], F32, tag="rz")
            nc.tensor.matmul(ps96[0:96, 0, :], W1a[:], anb[g][:], start=True, stop=True)
            nc.scalar.activation(hid[g][0:96, :], ps96[0:96, 0, :], AF.Relu)
            psh = ppg.tile([H, 2, BH], F32, tag="nh")
            nc.tensor.matmul(psh[:, 0, :], W2a[:], hid[g][:], start=True, stop=True)
            nc.scalar.activation(hbuf[g][0][:], psh[:, 0, :], AF.Identity)
            pool_eng.tensor_copy(hbb[g][0][0:H, :], hbuf[g][0][:])

        hk = [0, 0]  # current h ring index per half

        # one GRU tail: consumes psum tiles -> writes h_new (rows 0:H of buf)
        def gru_tail(g, ps_rz, ps_nh, eng):
            h_old = hbuf[g][hk[g]]
            h_new = hbuf[g][hk[g] ^ 1]
            hb_new = hbb[g][hk[g] ^ 1]
            th_rz = gp.tile([H, 2, BH], F32, tag=f"trz{g}")
            nc.scalar.activation(th_rz[:], ps_rz[:], AF.Tanh, scale=0.5)
            tmp = gq.tile([H, BH], F32, tag=f"gt{g}", name=f"t1{g}")
            nc.vector.scalar_tensor_tensor(tmp[:], th_rz[:, 0, :], 1.0, ps_nh[:, 1, :], ALU.add, ALU.mult)
            pre = gq.tile([H, BH], F32, tag=f"gt{g}", name=f"t2{g}")
            nc.vector.scalar_tensor_tensor(pre[:], tmp[:], 0.5, ps_nh[:, 0, :], ALU.mult, ALU.add)
            n_ = gq.tile([H, BH], F32, tag=f"gt{g}", name=f"t3{g}")
            nc.scalar.activation(n_[:], pre[:], AF.Tanh)
            d_ = gq.tile([H, BH], F32, tag=f"gt{g}", name=f"t4{g}")
            eng.tensor_sub(d_[:], n_[:], h_old[0:H, :])
            v1 = gq.tile([H, BH], F32, tag=f"gt{g}", name=f"t5{g}")
            eng.scalar_tensor_tensor(v1[:], th_rz[:, 1, :], -1.0, d_[:], ALU.add, ALU.mult)
            eng.scalar_tensor_tensor(h_new[:], v1[:], -0.5, h_old[:], ALU.mult, ALU.add)
            nc.scalar.copy(hb_new[0:H, :], h_new[:])
            hk[g] ^= 1
            return hb_new

        # ---------------- phase B: encoder ----------------
        for t in range(kt):
            e_rz, e_nh, e_old, e_new = {}, {}, {}, {}
            for g in (0, 1):
                xt = xtb[g][t % 2]
                nc.sync.dma_start(out=xt[0:ENC, :], in_=enc_d[t][:, gsl(g)])
                hcb = hbb[g][hk[g]]
                ps_rz = ppg.tile([H, 2, BH], F32, tag="rz", name="ps_rz")
                nc.tensor.matmul(ps_rz[:, 0, :], Wiea[:, 0:H], xt[:], start=True, stop=False)
                nc.tensor.matmul(ps_rz[:, 0, :], Whea[:, 0:H], hcb[:], start=False, stop=True)
                nc.tensor.matmul(ps_rz[:, 1, :], Wiea[:, H:2 * H], xt[:], start=True, stop=False)
                nc.tensor.matmul(ps_rz[:, 1, :], Whea[:, H:2 * H], hcb[:], start=False, stop=True)
                ps_nh = ppg.tile([H, 2, BH], F32, tag="nh", name="ps_nh")
                nc.tensor.matmul(ps_nh[:, 0, :], Wiea[:, 2 * H:3 * H], xt[:], start=True, stop=True)
                nc.tensor.matmul(ps_nh[:, 1, :], Whea[:, 2 * H:3 * H], hcb[:], start=True, stop=True)
                e_rz[g], e_nh[g] = ps_rz, ps_nh
                e_old[g] = hbuf[g][hk[g]]
                e_new[g] = hbuf[g][hk[g] ^ 1]
                hk[g] ^= 1

            # interleaved GRU tails (both halves) to fill ACT<->DVE gaps
            trz, tmp, pre, n_, d_, v1 = {}, {}, {}, {}, {}, {}
            for g in (0, 1):
                trz[g] = gp.tile([H, 2, BH], F32, tag=f"trz{g}", name="trz")
                nc.scalar.activation(trz[g][:], e_rz[g][:], AF.Tanh, scale=0.5)
            for g in (0, 1):
                tmp[g] = gq.tile([H, BH], F32, tag=f"gt{g}", name="tmp")
                nc.vector.scalar_tensor_tensor(tmp[g][:], trz[g][:, 0, :], 1.0, e_nh[g][:, 1, :], ALU.add, ALU.mult)
            for g in (0, 1):
                pre[g] = gq.tile([H, BH], F32, tag=f"gt{g}", name="pre")
                nc.vector.scalar_tensor_tensor(pre[g][:], tmp[g][:], 0.5, e_nh[g][:, 0, :], ALU.mult, ALU.add)
            for g in (0, 1):
                n_[g] = gq.tile([H, BH], F32, tag=f"gt{g}", name="n_")
                nc.scalar.activation(n_[g][:], pre[g][:], AF.Tanh)
            for g in (0, 1):
                d_[g] = gq.tile([H, BH], F32, tag=f"gt{g}", name="d_")
                pool_eng.tensor_sub(d_[g][:], n_[g][:], e_old[g][:])
            for g in (0, 1):
                v1[g] = gq.tile([H, BH], F32, tag=f"gt{g}", name="v1")
                nc.vector.scalar_tensor_tensor(v1[g][:], trz[g][:, 1, :], -1.0, d_[g][:], ALU.add, ALU.mult)
            for g in (0, 1):
                nc.vector.scalar_tensor_tensor(e_new[g][:], v1[g][:], -0.5, e_old[g][:], ALU.mult, ALU.add)

            for g in (0, 1):
                # bf16 mirror of the new hidden state (matmul operand + encb)
                hbm = hbb[g][hk[g]]
                nc.scalar.copy(hbm[0:H, :], e_new[g][:])
                # Uo[:, t, g] = U_aug @ h_new_aug  (bf16, bias folded)
                ps_uo = ppsc.tile([H, BH], F32, tag="sc", name="ps_uo")
                nc.tensor.matmul(ps_uo[:], UWa[:], hbm[:], start=True, stop=True)
                nc.scalar.copy(Uo[:, t, gsl(g)], ps_uo[:])

                # encb[:, 2g+cl, :, t] = h_new.T chunks (bf16)
                ptr = pptr.tile([128, 2, 128], BF16, tag="trb", name="ptr")
                for cl in (0, 1):
                    nc.tensor.transpose(ptr[0:128, cl, 0:H], hbm[0:H, cl * 128:(cl + 1) * 128], idb[0:H, 0:H])
                    nc.vector.tensor_copy(encb[:, 2 * g + cl, :, t], ptr[0:128, cl, 0:H])

        # ---------------- phase C: decoder ----------------
        # initial ground truth -> row 0 of first dxt buffers
        for g in (0, 1):
            nc.sync.dma_start(out=dxb[g][0][0:1, :], in_=enc_d[T - 1, 0:1, gsl(g)])

        prev_scr = [cp.tile([1, BH], BF16, tag=f"pv{g}", name=f"pv{g}") for g in (0, 1)]

        dec_state = {}

        def dec_A(s, g):
            dxt = dxb[g][s % 2]
            nc.sync.dma_start(out=dxt[1:DEC + 1, :], in_=dec_d[s][:, gsl(g)])
            h_cur = hbb[g][hk[g]]
            eeL = sp.tile([128, 2, 32], BF16, tag=f"eL{g}", name="eeL")
            eeH = sp.tile([128, 2, 24], BF16, tag=f"eH{g}", name="eeH")

            # Wh = Wl_aug @ h_aug (bias folded), -> bf16 [H,1,BH]
            ps_wh = ppsc.tile([H, BH], F32, tag="sc", name="ps_wh")
            nc.tensor.matmul(ps_wh[:], WlWa[:], h_cur[:], start=True, stop=True)
            wh = sp.tile([H, 1, BH], BF16, tag=f"wh{g}", name="wh")
            nc.scalar.copy(wh[:, 0, :], ps_wh[:])

            def emit_xq(q):
                xq = sp.tile([H, TQS, BH], BF16, tag=f"xq{g}", name="xq")
                xq_eng = nc.vector if (q < k_xqd or not k_pool) else pool_eng
                xq_eng.tensor_add(xq[:], Uo[:, q * TQS:(q + 1) * TQS, gsl(g)],
                                  wh[:].broadcast_to((H, TQS, BH)))
                nc.scalar.activation(xq[:], xq[:], AF.Tanh)
                xr = xq[:].rearrange("h t b -> h b t")
                pssc = ppsc.tile([128, 2, BBLK, TQS], F32, tag="sc", name="pssc")
                for cl in (0, 1):
                    for j in range(4):
                        b0 = cl * 128 + j * BBLK
                        nc.tensor.matmul(pssc[BBLK * j:BBLK * j + 1, cl, :, :], Vb[:],
                                         xr[:, b0:b0 + BBLK, :], start=True, stop=True,
                                         tile_position=(0, BBLK * j))
                return pssc

            def emit_evac(q, pssc):
                # fused evacuation + exp: ACT reads the scores PSUM, writes
                # exp(scores) bf16 to SBUF; the strided DMA then lands e in
                # batch-partitioned layout
                sstg = sp.tile([128, 2, BBLK, TQS], BF16, tag=f"sg{g}", name="sstg")
                nc.scalar.activation(sstg[:], pssc[:], AF.Exp)
                tgt_e = eeL if q < 4 else eeH
                toff = q * TQS if q < 4 else (q - 4) * TQS
                for cl in (0, 1):
                    nc.sync.dma_start(out=tgt_e[:, cl, toff:toff + TQS],
                                      in_=sstg[0:128:BBLK, cl, :, :])

            # stagger the PSUM evacuation one q behind the xq/tanh/scores
            # emission so no engine stalls on the ACT+PE round trip
            pending = None
            for q in range(NQ):
                pssc = emit_xq(q)
                if pending is not None:
                    emit_evac(q - 1, pending)
                pending = pssc
            emit_evac(NQ - 1, pending)
            dec_state[(s, g, 'ee')] = (eeL, eeH)
            dec_state[(s, g, 'h')] = h_cur
            dec_state[(s, g, 'dxt')] = dxt

        def dec_B(s, g):
            eeL, eeH = dec_state.pop((s, g, 'ee'))
            # attn = sum_t e * encb / sum_t e  (all on DVE; Pool holds xq work)
            attn_h = sp.tile([H, BH], BF16, tag=f"ah{g}", name="attn_h")
            ptr = pptr.tile([128, 2, 128], BF16, tag="trb", name="ptr")
            araw, inv = {}, None
            for cl in (0, 1):
                P1 = pq.tile([128, H, 32], BF16, tag="Pa", name="P1")
                P2 = pq.tile([128, H, 24], BF16, tag="Pb", name="P2")
                nc_c = 2 * g + cl
                nc.vector.tensor_mul(P1[:], encb[:, nc_c, :, 0:32],
                                     eeL[:, cl:cl + 1, :].broadcast_to((128, H, 32)))
                m2_eng = nc.gpsimd if os.environ.get("K_M2", "dve") == "pool" else nc.vector
                m2_eng.tensor_mul(P2[:], encb[:, nc_c, :, 32:56],
                                  eeH[:, cl:cl + 1, :].broadcast_to((128, H, 24)))
                (nc.gpsimd if (k_merge == 'pool' or (k_merge == 'split' and cl == 1)) else nc.vector).tensor_add(P1[:, :, 0:24], P1[:, :, 0:24], P2[:])
                nc.vector.tensor_add(P1[:, :, 0:16], P1[:, :, 0:16], P1[:, :, 16:32])
                nc.vector.tensor_add(P1[:, :, 0:8], P1[:, :, 0:8], P1[:, :, 8:16])
                nc.vector.tensor_add(P1[:, :, 0:4], P1[:, :, 0:4], P1[:, :, 4:8])
                araw[cl] = gp.tile([128, H], F32, tag=f"ar{g}", name="araw")
                nc.vector.tensor_reduce(araw[cl][:], P1[:, :, 0:4], axis=AX.X, op=ALU.add)
                if cl == 0:
                    seL = sp.tile([128, 2], F32, tag=f"sL{g}", name="seL")
                    nc.vector.tensor_reduce(seL[:], eeL[:], axis=AX.X, op=ALU.add)
                    seH = sp.tile([128, 2], F32, tag=f"sH{g}", name="seH")
                    nc.vector.tensor_reduce(seH[:], eeH[:], axis=AX.X, op=ALU.add)
                    se = sp.tile([128, 2], F32, tag=f"se{g}", name="se")
                    nc.vector.tensor_add(se[:], seL[:], seH[:])
                    inv = sp.tile([128, 2], F32, tag=f"iv{g}", name="inv")
                    nc.vector.reciprocal(inv[:], se[:])
            for cl in (0, 1):
                anrm = gp.tile([128, H], BF16, tag=f"an{g}", name="anrm")
                nc.vector.tensor_scalar_mul(anrm[:], araw[cl][:], inv[:, cl:cl + 1])
                nc.tensor.transpose(ptr[0:H, cl, 0:128], anrm[:], idb[:])
                nc.scalar.copy(attn_h[:, cl * 128:(cl + 1) * 128], ptr[0:H, cl, 0:128])
            dec_state[(s, g, 'attn')] = attn_h

        def dec_BC(s, g):
            dec_B(s, g)
            attn_h = dec_state.pop((s, g, 'attn'))
            h_cur = dec_state.pop((s, g, 'h'))
            dxt = dec_state.pop((s, g, 'dxt'))
            # decoder GRU (biases folded into Wda/Whda ones-rows)
            ps_rz = ppg.tile([H, 2, BH], F32, tag="rz", name="ps_rz")
            nc.tensor.matmul(ps_rz[:, 0, :], Wda[:, 0:H], dxt[:], start=True, stop=False)
            nc.tensor.matmul(ps_rz[:, 0, :], Whda[:, 0:H], h_cur[:], start=False, stop=False)
            nc.tensor.matmul(ps_rz[:, 0, :], WihAb[:, 0:H], attn_h[:], start=False, stop=True)
            nc.tensor.matmul(ps_rz[:, 1, :], Wda[:, H:2 * H], dxt[:], start=True, stop=False)
            nc.tensor.matmul(ps_rz[:, 1, :], Whda[:, H:2 * H], h_cur[:], start=False, stop=False)
            nc.tensor.matmul(ps_rz[:, 1, :], WihAb[:, H:2 * H], attn_h[:], start=False, stop=True)
            ps_nh = ppg.tile([H, 2, BH], F32, tag="nh", name="ps_nh")
            nc.tensor.matmul(ps_nh[:, 0, :], Wda[:, 2 * H:3 * H], dxt[:], start=True, stop=False)
            nc.tensor.matmul(ps_nh[:, 0, :], WihAb[:, 2 * H:3 * H], attn_h[:], start=False, stop=True)
            nc.tensor.matmul(ps_nh[:, 1, :], Whda[:, 2 * H:3 * H], h_cur[:], start=True, stop=True)
            hb_new = gru_tail(g, ps_rz, ps_nh, nc.vector)

            # out_s = h2o_aug @ h_new_aug -> DRAM, also next prev row
            ps_o = ppsc.tile([1, BH], F32, tag="sc", name="ps_o")
            nc.tensor.matmul(ps_o[:], h2oa[:], hb_new[:], start=True, stop=True)
            tgt = dxb[g][(s + 1) % 2][0:1, :] if s + 1 < S else prev_scr[g][:]
            nc.scalar.activation(tgt, ps_o[:], AF.Identity)
            nc.sync.dma_start(out=out_d[s, gsl(g)], in_=tgt)

        # antiphase software pipeline: while half 0 runs its scores stage
        # (Pool/ACT/PE), DVE chews half 1's attention math, and vice versa
        for s in range(ks):
            if k_order == 0:
                dec_A(s, 0)
                if s >= 1:
                    dec_BC(s - 1, 1)
                dec_A(s, 1)
                dec_BC(s, 0)
            elif k_order == 1:
                if s >= 1:
                    dec_BC(s - 1, 1)
                dec_A(s, 0)
                dec_A(s, 1)
                dec_BC(s, 0)
            else:
                dec_A(s, 0)
                if s >= 1:
                    dec_BC(s - 1, 1)
                dec_BC(s, 0)
                dec_A(s, 1)
        if ks >= 1:
            dec_BC(ks - 1, 1)
    _split_excess_waits(nc)
    return nc


def _host_inputs(inputs):
    import ml_dtypes
    bf16 = ml_dtypes.bfloat16
    f = lambda a: np.ascontiguousarray(np.asarray(a, dtype=np.float32).astype(bf16))

    def aug(wT, bias_rows):
        # wT: [in, out_cols]; append rows carrying biases
        return np.concatenate([wT] + [b.reshape(1, -1) for b in bias_rows], axis=0)

    ebih, ebhh = np.asarray(inputs["enc_bih"]), np.asarray(inputs["enc_bhh"])
    dbih, dbhh = np.asarray(inputs["dec_bih"]), np.asarray(inputs["dec_bhh"])
    dWih = np.asarray(inputs["dec_Wih"])  # [3H, 1+DEC+H]

    shared = {
        "W1a": f(aug(np.asarray(inputs["s2h_W1"]).T, [np.asarray(inputs["s2h_b1"])])),
        "W2a": f(aug(np.asarray(inputs["s2h_W2"]).T, [np.asarray(inputs["s2h_b2"])])),
        "Wiea": f(aug(np.asarray(inputs["enc_Wih"]).T, [ebih])),
        "Whea": f(aug(np.asarray(inputs["enc_Whh"]).T, [ebhh])),
        # decoder input block: [prev(1); dec(15); bias(1)] rows
        "Wda": f(np.concatenate([dWih[:, 0:1].T, dWih[:, 1:1 + DEC].T,
                                 dbih.reshape(1, -1)], axis=0)),
        "WihA": f(dWih[:, 1 + DEC:].T),
        "Whda": f(aug(np.asarray(inputs["dec_Whh"]).T, [dbhh])),
        "UWa": f(aug(np.asarray(inputs["U_W"]).T, [np.asarray(inputs["U_b"])])),
        "WlWa": f(aug(np.asarray(inputs["Wl_W"]).T, [np.asarray(inputs["Wl_b"])])),
        "h2oa": f(aug(np.asarray(inputs["h2o_W"]).T, [np.asarray(inputs["h2o_b"])])),
        "Vw": f(np.asarray(inputs["V_W"]).reshape(H, 1)),
        "ident": f(np.eye(128)),
        "ones": f(np.ones((1, BS), dtype=np.float32)),
    }

    enc = np.asarray(inputs["encoder_data"])   # [T, B, ENC]
    dec = np.asarray(inputs["decoder_data"])   # [S, B, DEC]
    ann = np.asarray(inputs["ann_data"])       # [B, ANN]
    maps = []
    for i in range(NCORES):
        sl = slice(i * BS, (i + 1) * BS)
        m = dict(shared)
        m["enc"] = f(enc[:, sl, :].transpose(0, 2, 1))
        m["dec"] = f(dec[:, sl, :].transpose(0, 2, 1))
        m["ann"] = f(ann[sl, :].T)
        maps.append(m)
    return maps


def kernel(**inputs) -> np.ndarray:
    from concourse.bass_utils import run_bass_kernel_spmd
    if "nc" not in _CACHE:
        _CACHE["nc"] = _build_program()
    nc = _CACHE["nc"]
    maps = _host_inputs(inputs)
    import os
    kw = {}
    if os.environ.get("KERNEL_TRACE") == "1":
        kw = dict(trace=True, trace_cores=[0])
    res = run_bass_kernel_spmd(nc, maps, list(range(NCORES)), **kw)
    _CACHE["last_res"] = res
    outs = [np.asarray(res.results[i]["out"]).astype(np.float32) for i in range(NCORES)]
    full = np.stack(outs, axis=1)              # [S, 8, 512]
    return full.reshape(S, B, 1)
